# revision 19
# baseline (speedup 1.0000x reference)
"""DKM (differentiable k-means with learned-MLP distance) Trainium2 kernel.

Data-parallel over N across 8 NeuronCores. Each core computes, for its
512-row shard of X:
  fxT[l, n]  = (X @ W1x.T).T                      (PE, f32r)
  fcbT[l, k] = (C @ W1c.T).T + b1                 (PE + DVE)
  h[l, n]    = relu(fxT + fcbT[:, k])  per (k, ltile)   (ACT/DVE split)
  P[k, n]    = sum_l w2[l] * h[l, n]              (PE, f32r, PSUM accum)
  a[n, k]    = softmax_k(-tanh(P + b2) / TEMP)    (ACT tanh/exp + PE transpose + DVE)
  S_part     = [a.T @ X | a.T @ 1]                (PE, f32r)
Host reduces S_part over cores and forms C_new = S[:, :E] / (a_sum + eps).
"""

import sys

sys.path.insert(0, "/opt/trn_rl_repo")

import numpy as np

N, K, E, L1 = 4096, 64, 768, 512
NCORES = 8
NC = N // NCORES          # 512 rows per core
NT = NC // 128            # 4 n-tiles
ET = E // 128             # 6 e-tiles
LT = L1 // 128            # 4 l-tiles
TEMP = 0.05
EPS = 1e-6
THRESHOLD = 1e-4

# every ACT_EVERY-th h tile is produced on the scalar engine, rest on DVE
ACT_EVERY = 3

_cached = {}


def _build():
    import concourse.bass as bass
    import concourse.tile as tile
    import concourse.mybir as mybir
    from concourse import bacc
    from concourse.masks import make_identity

    f32 = mybir.dt.float32
    f32r = mybir.dt.float32r
    f16 = mybir.dt.float16
    AF = mybir.ActivationFunctionType
    OP = mybir.AluOpType

    nc = bacc.Bacc("TRN2", target_bir_lowering=False, debug=False, num_devices=NCORES)

    xt = nc.dram_tensor("xt", [E, NC], f32r, kind="ExternalInput")
    xn = nc.dram_tensor("xn", [NC, E], f32r, kind="ExternalInput")
    w1xt = nc.dram_tensor("w1xt", [E, L1], f32r, kind="ExternalInput")
    w1ct = nc.dram_tensor("w1ct", [E, L1], f32r, kind="ExternalInput")
    ct = nc.dram_tensor("ct", [E, K], f32r, kind="ExternalInput")
    b1t = nc.dram_tensor("b1t", [128, LT], f32, kind="ExternalInput")
    b2b = nc.dram_tensor("b2b", [128, 1], f32, kind="ExternalInput")
    w2zin = nc.dram_tensor("w2z", [128, LT * (2 * K + 1)], f16, kind="ExternalInput")
    a_out = nc.dram_tensor("a_out", [NC, K], f32r, kind="ExternalOutput")
    s_out = nc.dram_tensor("s_out", [K, E], f32, kind="ExternalOutput")

    with tile.TileContext(nc) as tc:
        with (
            tc.tile_pool(name="persist", bufs=1) as persist,
            tc.tile_pool(name="hpool", bufs=20) as hpool,
            tc.tile_pool(name="epool", bufs=4) as epool,
            tc.tile_pool(name="pp_fx", bufs=4, space="PSUM") as pp_fx,
            tc.tile_pool(name="pp_fc", bufs=2, space="PSUM") as pp_fc,
            tc.tile_pool(name="pp_P", bufs=1, space="PSUM") as pp_P,
            tc.tile_pool(name="pp_tt", bufs=1, space="PSUM") as pp_tt,
        ):
            # ---- ACT table warmup: force the exp/tanh/relu set early ----
            warm = persist.tile([1, 1], f32, tag="warm")
            nc.vector.memset(warm, 0.0)
            nc.scalar.activation(out=warm, in_=warm, func=AF.Exp)
            nc.scalar.activation(out=warm, in_=warm, func=AF.Tanh)
            nc.scalar.activation(out=warm, in_=warm, func=AF.Relu)

            # ---- constants / small inputs (gpsimd queue; sync queue is for
            # the critical-path fx operands) ----
            b1t_sb = persist.tile([128, LT], f32, tag="b1t")
            b2b_full = persist.tile([128, 1], f32, tag="b2b")
            nc.gpsimd.dma_start(b1t_sb[:], b1t[:])
            nc.gpsimd.dma_start(b2b_full[:], b2b[:])
            ident = persist.tile([128, 128], f32, tag="ident")
            make_identity(nc, ident)
            w2z_sb = persist.tile([128, LT, 2 * K + 1], f16, tag="w2z")
            nc.gpsimd.dma_start(
                w2z_sb[:], w2zin[:].rearrange("p (t w) -> p t w", t=LT)
            )

            # ---- stream in the big operands, tile-granular for early start ----
            xt_sb = []
            w1xt_sb = []
            for e in range(ET):
                t1 = persist.tile([128, NC], f32r, tag=f"xt{e}")
                nc.sync.dma_start(t1[:], xt[e * 128 : (e + 1) * 128, :])
                xt_sb.append(t1)
                t2 = persist.tile([128, L1], f32r, tag=f"w1xt{e}")
                nc.scalar.dma_start(t2[:], w1xt[e * 128 : (e + 1) * 128, :])
                w1xt_sb.append(t2)
            w1ct_sb = []
            ct_sb = []
            for e in range(ET):
                t3 = persist.tile([128, L1], f32r, tag=f"w1ct{e}")
                nc.gpsimd.dma_start(t3[:], w1ct[e * 128 : (e + 1) * 128, :])
                w1ct_sb.append(t3)
                t4 = persist.tile([128, K], f32r, tag=f"ct{e}")
                nc.gpsimd.dma_start(t4[:], ct[e * 128 : (e + 1) * 128, :])
                ct_sb.append(t4)
            xn_sb = []
            for t in range(NT):
                t5 = persist.tile([128, E], f32r, tag=f"xn{t}")
                nc.scalar.dma_start(t5[:], xn[t * 128 : (t + 1) * 128, :])
                xn_sb.append(t5)

            # ---- fxT[l, n] = sum_e W1xT[e, l] * XT[e, n]  (4 l-tiles) ----
            fxs = []
            fxp = []
            for t in range(LT):
                ps = pp_fx.tile([128, NC], f32, tag="fx")
                for e in range(ET):
                    nc.tensor.matmul(
                        ps[:],
                        lhsT=w1xt_sb[e][:, t * 128 : (t + 1) * 128],
                        rhs=xt_sb[e][:],
                        start=(e == 0),
                        stop=(e == ET - 1),
                    )
                sb = persist.tile([128, NC], f16, tag=f"fxs{t}")
                nc.any.tensor_copy(out=sb[:], in_=ps[:])
                fxs.append(sb)
                fxp.append(ps)

            # ---- fc_nat[k, l] = sum_e CT[e, k] * W1cT[e, l]  (one wide
            # matmul per e-tile with the small CT as stationary), then
            # transpose 128-col blocks to fcbT[l, k] and add b1 ----
            fcn = pp_fc.tile([K, L1], f32, tag="fc")
            for e in range(ET):
                nc.tensor.matmul(
                    fcn[:],
                    lhsT=ct_sb[e][:],
                    rhs=w1ct_sb[e][:],
                    start=(e == 0),
                    stop=(e == ET - 1),
                )
            fcn_sb = persist.tile([K, L1], f32, tag="fcn")
            nc.scalar.copy(fcn_sb[:], fcn[:])
            fcb = []
            for t in range(LT):
                ft = pp_tt.tile([128, K], f32, tag="tt")
                nc.tensor.transpose(
                    ft[:], fcn_sb[:, t * 128 : (t + 1) * 128], ident[:K, :K]
                )
                sb = persist.tile([128, K], f32, tag=f"fcb{t}")
                nc.vector.tensor_scalar(
                    sb[:], ft[:], b1t_sb[:, t : t + 1], None, OP.add
                )
                fcb.append(sb)

            # ---- main loop: h = relu(fxT + fcb[:, k]);  P[k, :] += w2_t . h ----
            # Shifted-window stationary (host-built): w2z_sb[:, t] = [0_K | w2_t | 0_K]
            # so lhsT = w2z_sb[:, t, K-k : 2K-k] has w2_t in column k, zeros
            # elsewhere. Every matmul writes the full [K, NC] PSUM tile at base
            # partition 0 (HW requires 32-aligned output base partitions),
            # accumulating zeros into the other rows.
            P = pp_P.tile([K, NC], f32, tag="P")
            idx = 0
            for k in range(K):
                for t in range(LT):
                    h = hpool.tile([128, NC], f16, tag="h")
                    bias_ap = fcb[t][:, k : k + 1]
                    if idx % ACT_EVERY == 0:
                        nc.scalar.activation(
                            out=h[:], in_=fxp[t][:], func=AF.Relu, bias=bias_ap
                        )
                    else:
                        nc.vector.tensor_scalar(
                            h[:], fxs[t][:], bias_ap, 0.0, OP.add, OP.max
                        )
                    nc.tensor.matmul(
                        P[:],
                        lhsT=w2z_sb[:, t, K - k : 2 * K - k],
                        rhs=h[:],
                        start=(idx == 0),
                        stop=(idx == K * LT - 1),
                    )
                    idx += 1

            # ---- softmax over k:  a = exp(-tanh(P + b2)/TEMP) / rowsum ----
            tanh_sb = persist.tile([K, NC], f32, tag="tanh")
            nc.scalar.activation(
                out=tanh_sb[:], in_=P[:], func=AF.Tanh, bias=b2b_full[:K]
            )
            S1 = pp_fc.tile([K, 512], f32, tag="fc")
            S2 = pp_fc.tile([K, E - 512], f32, tag="fc")
            for t in range(NT):
                tt = pp_tt.tile([128, K], f32, tag="tt")
                nc.tensor.transpose(
                    tt[:], tanh_sb[:, t * 128 : (t + 1) * 128], ident[:K, :K]
                )
                e_sb = epool.tile([128, K], f32, tag="esb")
                nc.scalar.activation(
                    out=e_sb[:], in_=tt[:], func=AF.Exp, scale=-1.0 / TEMP
                )
                srow = epool.tile([128, 1], f32, tag="srow")
                nc.vector.tensor_reduce(
                    out=srow[:], in_=e_sb[:], axis=mybir.AxisListType.X, op=OP.add
                )
                rrow = epool.tile([128, 1], f32, tag="rrow")
                nc.vector.reciprocal(rrow[:], srow[:])
                a_sb = epool.tile([128, K], f32r, tag="asb")
                nc.vector.tensor_scalar_mul(a_sb[:], e_sb[:], rrow[:])
                nc.sync.dma_start(a_out[t * 128 : (t + 1) * 128, :], a_sb[:])

                # S += a_tile.T @ [X_tile | 1]
                nc.tensor.matmul(
                    S1[:],
                    lhsT=a_sb[:],
                    rhs=xn_sb[t][:, 0:512],
                    start=(t == 0),
                    stop=(t == NT - 1),
                )
                nc.tensor.matmul(
                    S2[:],
                    lhsT=a_sb[:],
                    rhs=xn_sb[t][:, 512:E],
                    start=(t == 0),
                    stop=(t == NT - 1),
                )

            ss = persist.tile([K, E], f32, tag="ss")
            nc.scalar.copy(ss[:, 0:512], S1[:])
            nc.vector.tensor_copy(out=ss[:, 512:E], in_=S2[:])
            nc.sync.dma_start(s_out[:], ss[:])

    nc.compile()
    return nc


def _get_nc():
    if "nc" not in _cached:
        _cached["nc"] = _build()
    return _cached["nc"]


def kernel(X, C_init, W1, b1, w2, b2):
    from concourse.bass_utils import run_bass_kernel_spmd

    X = np.ascontiguousarray(np.asarray(X, dtype=np.float32))
    C_init = np.ascontiguousarray(np.asarray(C_init, dtype=np.float32))
    W1 = np.asarray(W1, dtype=np.float32)
    b1 = np.asarray(b1, dtype=np.float32)
    w2 = np.asarray(w2, dtype=np.float32)
    b2f = float(np.asarray(b2, dtype=np.float32))

    w1xt = np.ascontiguousarray(W1[:, :E].T)          # [E, L1]
    w1ct = np.ascontiguousarray(W1[:, E:].T)          # [E, L1]
    ctm = np.ascontiguousarray(C_init.T)              # [E, K]
    b1tm = np.ascontiguousarray(b1.reshape(LT, 128).T)  # [128, LT]
    b2bm = np.full((128, 1), b2f, dtype=np.float32)
    w2zm = np.zeros((128, LT, 2 * K + 1), dtype=np.float16)
    for t in range(LT):
        w2zm[:, t, K] = w2[t * 128 : (t + 1) * 128].astype(np.float16)
    w2zm = np.ascontiguousarray(w2zm.reshape(128, LT * (2 * K + 1)))


    in_maps = []
    for c in range(NCORES):
        xs = X[c * NC : (c + 1) * NC, :]
        in_maps.append(
            {
                "xt": np.ascontiguousarray(xs.T),
                "xn": np.ascontiguousarray(xs),
                "w1xt": w1xt,
                "w1ct": w1ct,
                "ct": ctm,
                "b1t": b1tm,
                "b2b": b2bm,
                "w2z": w2zm,
            }
        )

    nc = _get_nc()
    res = run_bass_kernel_spmd(nc, in_maps, core_ids=list(range(NCORES)))

    a = np.concatenate([r["a_out"] for r in res.results], axis=0)
    S = np.sum(np.stack([r["s_out"] for r in res.results]), axis=0)
    a_sum = a.sum(axis=0, dtype=np.float32) + EPS
    C_new = S / a_sum[:, None]
    diff = np.abs(C_new - C_init).sum()
    C = C_new if diff > THRESHOLD else C_init
    return C.astype(np.float32), a.astype(np.float32)


# revision 20
# speedup vs baseline: 1.0506x; 1.0506x over previous
"""DKM (differentiable k-means with learned-MLP distance) Trainium2 kernel.

Data-parallel over N across 8 NeuronCores. Each core computes, for its
512-row shard of X:
  fxT[l, n]  = (X @ W1x.T).T                      (PE, f32r)
  fcbT[l, k] = (C @ W1c.T).T + b1                 (PE + DVE)
  h[l, n]    = relu(fxT + fcbT[:, k])  per (k, ltile)   (ACT/DVE split)
  P[k, n]    = sum_l w2[l] * h[l, n]              (PE, f32r, PSUM accum)
  a[n, k]    = softmax_k(-tanh(P + b2) / TEMP)    (ACT tanh/exp + PE transpose + DVE)
  S_part     = [a.T @ X | a.T @ 1]                (PE, f32r)
Host reduces S_part over cores and forms C_new = S[:, :E] / (a_sum + eps).
"""

import sys

sys.path.insert(0, "/opt/trn_rl_repo")

import numpy as np

N, K, E, L1 = 4096, 64, 768, 512
NCORES = 8
NC = N // NCORES          # 512 rows per core
NT = NC // 128            # 4 n-tiles
ET = E // 128             # 6 e-tiles
LT = L1 // 128            # 4 l-tiles
TEMP = 0.05
EPS = 1e-6
THRESHOLD = 1e-4

# every ACT_EVERY-th h tile is produced on the scalar engine, rest on DVE
ACT_EVERY = 3

_cached = {}


def _build():
    import concourse.bass as bass
    import concourse.tile as tile
    import concourse.mybir as mybir
    from concourse import bacc
    from concourse.masks import make_identity

    f32 = mybir.dt.float32
    f32r = mybir.dt.float32r
    f16 = mybir.dt.float16
    AF = mybir.ActivationFunctionType
    OP = mybir.AluOpType

    nc = bacc.Bacc("TRN2", target_bir_lowering=False, debug=False, num_devices=NCORES)

    xt = nc.dram_tensor("xt", [E, NC], f32r, kind="ExternalInput")
    xn = nc.dram_tensor("xn", [NC, E], f32r, kind="ExternalInput")
    w1xt = nc.dram_tensor("w1xt", [E, L1], f32r, kind="ExternalInput")
    w1ct = nc.dram_tensor("w1ct", [E, L1], f32r, kind="ExternalInput")
    ct = nc.dram_tensor("ct", [E, K], f32r, kind="ExternalInput")
    b1t = nc.dram_tensor("b1t", [128, LT], f32, kind="ExternalInput")
    b2b = nc.dram_tensor("b2b", [128, 1], f32, kind="ExternalInput")
    w2zin = nc.dram_tensor("w2z", [128, LT * (2 * K + 1)], f16, kind="ExternalInput")
    a_out = nc.dram_tensor("a_out", [NC, K], f32r, kind="ExternalOutput")
    s_out = nc.dram_tensor("s_out", [K, E], f32, kind="ExternalOutput")

    with tile.TileContext(nc) as tc:
        with (
            tc.tile_pool(name="persist", bufs=1) as persist,
            tc.tile_pool(name="hpool", bufs=20) as hpool,
            tc.tile_pool(name="epool", bufs=4) as epool,
            tc.tile_pool(name="pp_fx", bufs=2, space="PSUM") as pp_fx,
            tc.tile_pool(name="pp_fc", bufs=1, space="PSUM") as pp_fc,
            tc.tile_pool(name="pp_P", bufs=1, space="PSUM") as pp_P,
            tc.tile_pool(name="pp_tt", bufs=2, space="PSUM") as pp_tt,
            tc.tile_pool(name="pp_s", bufs=1, space="PSUM") as pp_s,
        ):
            # ---- ACT table warmup: force the exp/tanh/relu set early ----
            warm = persist.tile([1, 1], f32, tag="warm")
            nc.vector.memset(warm, 0.0)
            nc.scalar.activation(out=warm, in_=warm, func=AF.Exp)
            nc.scalar.activation(out=warm, in_=warm, func=AF.Tanh)
            nc.scalar.activation(out=warm, in_=warm, func=AF.Relu)

            # ---- constants / small inputs (gpsimd queue; sync queue is for
            # the critical-path fx operands) ----
            b1t_sb = persist.tile([128, LT], f32, tag="b1t")
            b2b_full = persist.tile([128, 1], f32, tag="b2b")
            nc.gpsimd.dma_start(b1t_sb[:], b1t[:])
            nc.gpsimd.dma_start(b2b_full[:], b2b[:])
            ident = persist.tile([128, 128], f32, tag="ident")
            make_identity(nc, ident)
            w2z_sb = persist.tile([128, LT, 2 * K + 1], f16, tag="w2z")
            nc.gpsimd.dma_start(
                w2z_sb[:], w2zin[:].rearrange("p (t w) -> p t w", t=LT)
            )

            # ---- stream in the big operands, tile-granular for early start ----
            xt_sb = []
            w1xt_sb = []
            for e in range(ET):
                t1 = persist.tile([128, NC], f32r, tag=f"xt{e}")
                nc.sync.dma_start(t1[:], xt[e * 128 : (e + 1) * 128, :])
                xt_sb.append(t1)
                t2 = persist.tile([128, L1], f32r, tag=f"w1xt{e}")
                nc.sync.dma_start(t2[:], w1xt[e * 128 : (e + 1) * 128, :])
                w1xt_sb.append(t2)
            w1ct_sb = []
            ct_sb = []
            for e in range(ET):
                t3 = persist.tile([128, L1], f32r, tag=f"w1ct{e}")
                nc.gpsimd.dma_start(t3[:], w1ct[e * 128 : (e + 1) * 128, :])
                w1ct_sb.append(t3)
                t4 = persist.tile([128, K], f32r, tag=f"ct{e}")
                nc.gpsimd.dma_start(t4[:], ct[e * 128 : (e + 1) * 128, :])
                ct_sb.append(t4)
            xn_sb = []
            for t in range(NT):
                t5 = persist.tile([128, E], f32r, tag=f"xn{t}")
                nc.gpsimd.dma_start(t5[:], xn[t * 128 : (t + 1) * 128, :])
                xn_sb.append(t5)

            # ---- fxT[l, n] = sum_e W1xT[e, l] * XT[e, n]  (4 l-tiles) ----
            fxs = []
            for t in range(LT):
                ps = pp_fx.tile([128, NC], f32, tag="fx")
                for e in range(ET):
                    nc.tensor.matmul(
                        ps[:],
                        lhsT=w1xt_sb[e][:, t * 128 : (t + 1) * 128],
                        rhs=xt_sb[e][:],
                        start=(e == 0),
                        stop=(e == ET - 1),
                    )
                sb = persist.tile([128, NC], f16, tag=f"fxs{t}")
                nc.any.tensor_copy(out=sb[:], in_=ps[:])
                fxs.append(sb)

            # ---- fc_nat[k, l] = sum_e CT[e, k] * W1cT[e, l]  (one wide
            # matmul per e-tile with the small CT as stationary), then
            # transpose 128-col blocks to fcbT[l, k] and add b1 ----
            fcn = pp_fc.tile([K, L1], f32, tag="fc")
            for e in range(ET):
                nc.tensor.matmul(
                    fcn[:],
                    lhsT=ct_sb[e][:],
                    rhs=w1ct_sb[e][:],
                    start=(e == 0),
                    stop=(e == ET - 1),
                )
            fcn_sb = persist.tile([K, L1], f32, tag="fcn")
            nc.scalar.copy(fcn_sb[:], fcn[:])
            fcb = []
            for t in range(LT):
                ft = pp_tt.tile([128, K], f32, tag="tt")
                nc.tensor.transpose(
                    ft[:], fcn_sb[:, t * 128 : (t + 1) * 128], ident[:K, :K]
                )
                sb = persist.tile([128, K], f32, tag=f"fcb{t}")
                nc.vector.tensor_scalar(
                    sb[:], ft[:], b1t_sb[:, t : t + 1], None, OP.add
                )
                fcb.append(sb)

            # ---- main loop: h = relu(fxT + fcb[:, k]);  P[k, :] += w2_t . h ----
            # Shifted-window stationary (host-built): w2z_sb[:, t] = [0_K | w2_t | 0_K]
            # so lhsT = w2z_sb[:, t, K-k : 2K-k] has w2_t in column k, zeros
            # elsewhere. Every matmul writes the full [K, NC] PSUM tile at base
            # partition 0 (HW requires 32-aligned output base partitions),
            # accumulating zeros into the other rows.
            P = pp_P.tile([K, NC], f32, tag="P")
            idx = 0
            for k in range(K):
                for t in range(LT):
                    h = hpool.tile([128, NC], f16, tag="h")
                    bias_ap = fcb[t][:, k : k + 1]
                    if idx % ACT_EVERY == 0:
                        nc.scalar.activation(
                            out=h[:], in_=fxs[t][:], func=AF.Relu, bias=bias_ap
                        )
                    else:
                        nc.vector.tensor_scalar(
                            h[:], fxs[t][:], bias_ap, 0.0, OP.add, OP.max
                        )
                    nc.tensor.matmul(
                        P[:],
                        lhsT=w2z_sb[:, t, K - k : 2 * K - k],
                        rhs=h[:],
                        start=(idx == 0),
                        stop=(idx == K * LT - 1),
                    )
                    idx += 1

            # ---- softmax over k:  a = exp(-tanh(P + b2)/TEMP) / rowsum ----
            tanh_sb = persist.tile([K, NC], f32, tag="tanh")
            nc.scalar.activation(
                out=tanh_sb[:], in_=P[:], func=AF.Tanh, bias=b2b_full[:K]
            )
            S1 = pp_s.tile([K, 512], f32, tag="S1")
            S2 = pp_s.tile([K, E - 512], f32, tag="S2")
            for t in range(NT):
                tt = pp_tt.tile([128, K], f32, tag="tt")
                nc.tensor.transpose(
                    tt[:], tanh_sb[:, t * 128 : (t + 1) * 128], ident[:K, :K]
                )
                e_sb = epool.tile([128, K], f32, tag="esb")
                nc.scalar.activation(
                    out=e_sb[:], in_=tt[:], func=AF.Exp, scale=-1.0 / TEMP
                )
                srow = epool.tile([128, 1], f32, tag="srow")
                nc.vector.tensor_reduce(
                    out=srow[:], in_=e_sb[:], axis=mybir.AxisListType.X, op=OP.add
                )
                rrow = epool.tile([128, 1], f32, tag="rrow")
                nc.vector.reciprocal(rrow[:], srow[:])
                a_sb = epool.tile([128, K], f32r, tag="asb")
                nc.vector.tensor_scalar_mul(a_sb[:], e_sb[:], rrow[:])
                nc.sync.dma_start(a_out[t * 128 : (t + 1) * 128, :], a_sb[:])

                # S += a_tile.T @ [X_tile | 1]
                nc.tensor.matmul(
                    S1[:],
                    lhsT=a_sb[:],
                    rhs=xn_sb[t][:, 0:512],
                    start=(t == 0),
                    stop=(t == NT - 1),
                )
                nc.tensor.matmul(
                    S2[:],
                    lhsT=a_sb[:],
                    rhs=xn_sb[t][:, 512:E],
                    start=(t == 0),
                    stop=(t == NT - 1),
                )

            ss = persist.tile([K, E], f32, tag="ss")
            nc.scalar.copy(ss[:, 0:512], S1[:])
            nc.vector.tensor_copy(out=ss[:, 512:E], in_=S2[:])
            nc.sync.dma_start(s_out[:], ss[:])

    nc.compile()
    return nc


def _get_nc():
    if "nc" not in _cached:
        _cached["nc"] = _build()
    return _cached["nc"]


def kernel(X, C_init, W1, b1, w2, b2):
    from concourse.bass_utils import run_bass_kernel_spmd

    X = np.ascontiguousarray(np.asarray(X, dtype=np.float32))
    C_init = np.ascontiguousarray(np.asarray(C_init, dtype=np.float32))
    W1 = np.asarray(W1, dtype=np.float32)
    b1 = np.asarray(b1, dtype=np.float32)
    w2 = np.asarray(w2, dtype=np.float32)
    b2f = float(np.asarray(b2, dtype=np.float32))

    w1xt = np.ascontiguousarray(W1[:, :E].T)          # [E, L1]
    w1ct = np.ascontiguousarray(W1[:, E:].T)          # [E, L1]
    ctm = np.ascontiguousarray(C_init.T)              # [E, K]
    b1tm = np.ascontiguousarray(b1.reshape(LT, 128).T)  # [128, LT]
    b2bm = np.full((128, 1), b2f, dtype=np.float32)
    w2zm = np.zeros((128, LT, 2 * K + 1), dtype=np.float16)
    for t in range(LT):
        w2zm[:, t, K] = w2[t * 128 : (t + 1) * 128].astype(np.float16)
    w2zm = np.ascontiguousarray(w2zm.reshape(128, LT * (2 * K + 1)))


    in_maps = []
    for c in range(NCORES):
        xs = X[c * NC : (c + 1) * NC, :]
        in_maps.append(
            {
                "xt": np.ascontiguousarray(xs.T),
                "xn": np.ascontiguousarray(xs),
                "w1xt": w1xt,
                "w1ct": w1ct,
                "ct": ctm,
                "b1t": b1tm,
                "b2b": b2bm,
                "w2z": w2zm,
            }
        )

    nc = _get_nc()
    res = run_bass_kernel_spmd(nc, in_maps, core_ids=list(range(NCORES)))

    a = np.concatenate([r["a_out"] for r in res.results], axis=0)
    S = np.sum(np.stack([r["s_out"] for r in res.results]), axis=0)
    a_sum = a.sum(axis=0, dtype=np.float32) + EPS
    C_new = S / a_sum[:, None]
    diff = np.abs(C_new - C_init).sum()
    C = C_new if diff > THRESHOLD else C_init
    return C.astype(np.float32), a.astype(np.float32)
